# revision 52
# baseline (speedup 1.0000x reference)
"""Angular-prototypical hard-mining loss on 8 Trainium2 cores (v2).

Host sorts rows by label -> same-label pairs cluster near the diagonal.
Each core gets a 1024-row slab and a column-ROTATED feats^T so its slab is
local columns [0,1024) -> one uniform SPMD program. Per 128-row m-tile,
1-2 aligned 512-col "strip" tiles hold all same-label columns
(host-verified); the rest are pure cross-label.

v2 engine plan (per [128,2048] column group, fp32r matmuls):
  PE:   2 fp32r matmuls per 512-tile into a 4-bank PSUM group; strip tiles
        get a third small bf16 one-hot-label matmul accumulating -30*same
        (includes the diagonal) straight into PSUM -> masked sim with no
        DVE mask work and no value-based diagonal detection.
  ACT:  one Exp over the whole group (scale=50, bias=-25), bf16 out to
        SBUF, fused accum_out = unthresholded neg-LSE partial.
  DVE:  row max via bf16 tensor_tensor(max) chains over the exp outputs
        (2x DVE mode); max_neg recovered on host as (ln(max_e)+25)/50.
        Strips: one tensor_tensor_reduce makes vs = sim + maskD (+30 off
        targets, +60 diag) AND its row min (-> min_pos).
  Pos phase (needs tp = max_neg + margin): one on-device Ln of the 8
        row-max columns, tp8 = lgE/50 + 0.6, then per strip one ACT Exp
        (scale=-2, bias=+1) and one scalar_tensor_tensor
        (vs < tp) * e_v with fused row-sum -> pos partial.

Per-row lgE/min/neg/pos partials go back to the host, which does
valid/log1p/loss/prec1 in f32 (order-invariant -> no un-sort needed).
"""
import os
import sys
import numpy as np

sys.path.insert(0, "/opt/trn_rl_repo")

USE_TTR = os.environ.get("K_TTR", "1") == "1"
USE_STT = os.environ.get("K_STT", "1") == "1"
USE_LN = os.environ.get("K_LN", "1") == "1"
USE_GRPACT = os.environ.get("K_GRPACT", "1") == "1"
USE_HOTMM = os.environ.get("K_HOTMM", "1") == "1"

B, D, NCORES, SLAB = 8192, 256, 8, 1024
P, NT, M_TILES, N_TILES = 128, 512, 8, 16
KHOT = 64
THRESH, MARGIN, SP, SN, EPS = 0.5, 0.1, 2.0, 50.0, 1e-5
MASK_NEG = -30.0   # added to same-label entries (incl diag) for the neg side
BIGV = 1e9
E_MARGIN = 148.4131591025766   # e^(SN*MARGIN): pos threshold in exp domain

LAST_EXEC_NS = None
LAST_TRACE = None

STRIP = {0: (15, 0), 1: (0,), 2: (0,), 3: (0, 1), 4: (0, 1),
         5: (1,), 6: (1,), 7: (1, 2)}
COV = {0: (-512, 512), 1: (0, 512), 2: (0, 512), 3: (0, 1024),
       4: (0, 1024), 5: (512, 1024), 6: (512, 1024), 7: (512, 1536)}
COMBOS = [(m, n) for m in range(M_TILES) for n in STRIP[m]]
NK = len(COMBOS)  # 12
POOL_M = frozenset()  # walrus: Pool supports only memset/DMA, no tensor ops


def _pos_phase(nc, m, K_IDX, vs_all, tp8, sp, bias_n, bias_p, posp, Act, Alu,
               BF16):
    from concourse import mybir
    X = mybir.AxisListType.X
    for n2 in STRIP[m]:
        k = K_IDX[(m, n2)]
        vs = vs_all[:, k * NT:(k + 1) * NT]
        ev = sp.tile([P, NT], BF16, name=f"ev{k}", tag="ev")
        nc.scalar.activation(out=ev[:], in_=vs, func=Act.Exp,
                             bias=bias_p[:], scale=-SP)
        junk = sp.tile([P, NT], BF16, name=f"junk{k}", tag="junk")
        if USE_STT:
            nc.vector.scalar_tensor_tensor(
                out=junk[:], in0=vs, scalar=tp8[:, m:m + 1], in1=ev[:],
                op0=Alu.is_lt, op1=Alu.mult, accum_out=posp[:, k:k + 1])
        else:
            msk = sp.tile([P, NT], BF16, name=f"msk{k}", tag="msk")
            nc.vector.tensor_scalar(out=msk[:], in0=vs,
                                    scalar1=tp8[:, m:m + 1], scalar2=None,
                                    op0=Alu.is_lt)
            nc.vector.tensor_tensor(out=junk[:], in0=msk[:], in1=ev[:],
                                    op=Alu.mult)
            nc.vector.reduce_sum(posp[:, k:k + 1], junk[:], axis=X)


def _loss_kernel(tc, outs, ins):
    from concourse import mybir
    from contextlib import ExitStack

    F32, BF16 = mybir.dt.float32, mybir.dt.bfloat16
    F32R = mybir.dt.float32r
    Alu, Act = mybir.AluOpType, mybir.ActivationFunctionType
    X = mybir.AxisListType.X
    nc = tc.nc
    fk0_d, fk1_d = ins["fk0"], ins["fk1"]
    maskD_d, rowhot_d, colhot_d = ins["maskD"], ins["rowhot"], ins["colhot"]
    K_IDX = {c: i for i, c in enumerate(COMBOS)}

    with ExitStack() as ctx:
        big = ctx.enter_context(tc.tile_pool(name="big", bufs=1))
        ep = ctx.enter_context(tc.tile_pool(name="ep", bufs=3))
        sp = ctx.enter_context(tc.tile_pool(name="sp", bufs=3))
        psp = ctx.enter_context(tc.tile_pool(name="psum", bufs=2, space="PSUM"))

        fk0 = big.tile([P, B], BF16)
        fk1 = big.tile([P, B], BF16)
        maskD = big.tile([P, NK * NT], BF16)
        rowhot = big.tile([KHOT, NK * P], BF16)
        colhot = big.tile([KHOT, NK * NT], BF16)
        vs_all = big.tile([P, NK * NT], F32)
        maxacc = [big.tile([P, NT], BF16, name=f"maxacc{m}", tag=f"mx{m}")
                  for m in range(M_TILES)]
        mxE = big.tile([P, M_TILES], F32)
        lg8 = big.tile([P, M_TILES], F32)
        tp8 = big.tile([P, M_TILES], F32)
        mnp = big.tile([P, NK], F32)
        negp = big.tile([P, 16 * M_TILES], F32)
        posp = big.tile([P, NK], F32)
        nc.vector.memset(negp[:], 0.0)
        bias_n = big.tile([P, 1], F32)
        bias_p = big.tile([P, 1], F32)
        bias_z = big.tile([P, 1], F32)
        nc.vector.memset(bias_n[:], -SN * THRESH)
        nc.vector.memset(bias_p[:], SP * THRESH)
        nc.vector.memset(bias_z[:], 0.0)

        # group 0 (column tiles 0-3) holds 11 of the 12 strips -> process it
        # LAST so the mask DMAs never gate the pipeline start. But cols
        # [0:1024) are every m-tile's stationary operand -> DMA them FIRST.
        G_ORDER = (1, 2, 3, 0)
        CH = 2048
        lhs = slice(0, SLAB)
        nc.sync.dma_start(fk0[:, lhs], fk0_d[:, lhs])
        nc.sync.dma_start(fk1[:, lhs], fk1_d[:, lhs])
        for i in (1, 2, 3):
            cs = slice(i * CH, (i + 1) * CH)
            nc.sync.dma_start(fk0[:, cs], fk0_d[:, cs])
            nc.sync.dma_start(fk1[:, cs], fk1_d[:, cs])
        # masks + trailing g0 columns go down the (idle) gpsimd SWDGE queue,
        # in parallel with the fk stream on the sync queue.
        nc.gpsimd.dma_start(maskD[:], maskD_d[:])
        nc.gpsimd.dma_start(rowhot[:], rowhot_d[:])
        nc.gpsimd.dma_start(colhot[:], colhot_d[:])
        g0r = slice(SLAB, CH)
        nc.gpsimd.dma_start(fk0[:, g0r], fk0_d[:, g0r])
        nc.gpsimd.dma_start(fk1[:, g0r], fk1_d[:, g0r])

        GW = 2048
        for gi, g in enumerate(G_ORDER):
            for m in range(M_TILES):
                mc = slice(m * P, (m + 1) * P)
                pt = psp.tile([P, GW], F32, tag="ps")
                for t in range(4):
                    n = g * 4 + t
                    ncs = slice(n * NT, (n + 1) * NT)
                    sl = pt[:, t * NT:(t + 1) * NT]
                    is_strip = (m, n) in K_IDX
                    nc.tensor.matmul(sl, fk0[:, mc], fk0[:, ncs],
                                     start=True, stop=False)
                    nc.tensor.matmul(sl, fk1[:, mc], fk1[:, ncs],
                                     start=False, stop=not is_strip)
                    if is_strip and USE_HOTMM:
                        k = K_IDX[(m, n)]
                        nc.tensor.matmul(sl, rowhot[:, k * P:(k + 1) * P],
                                         colhot[:, k * NT:(k + 1) * NT],
                                         start=False, stop=True)
                e = ep.tile([P, GW], BF16, tag="e")
                if USE_GRPACT:
                    nc.scalar.activation(
                        out=e[:], in_=pt[:], func=Act.Exp,
                        bias=bias_n[:], scale=SN,
                        accum_out=negp[:, m * 16 + g * 4:m * 16 + g * 4 + 1])
                else:
                    for t in range(4):
                        nc.scalar.activation(
                            out=e[:, t * NT:(t + 1) * NT],
                            in_=pt[:, t * NT:(t + 1) * NT], func=Act.Exp,
                            bias=bias_n[:], scale=SN,
                            accum_out=negp[:, m * 16 + g * 4 + t:
                                           m * 16 + g * 4 + t + 1])
                # row-max chain over the bf16 exps: pairwise tree fold of the
                # group (2x DVE mode on bf16), then max-accumulate into maxacc
                fa = ep.tile([P, 1024], BF16, tag="fa")
                nc.vector.tensor_tensor(out=fa[:], in0=e[:, 0:1024],
                                        in1=e[:, 1024:2048], op=Alu.max)
                if gi == 0:
                    nc.vector.tensor_tensor(out=maxacc[m][:], in0=fa[:, 0:NT],
                                            in1=fa[:, NT:1024], op=Alu.max)
                else:
                    fb = ep.tile([P, NT], BF16, tag="fb")
                    nc.vector.tensor_tensor(out=fb[:], in0=fa[:, 0:NT],
                                            in1=fa[:, NT:1024], op=Alu.max)
                    nc.vector.tensor_tensor(out=maxacc[m][:],
                                            in0=maxacc[m][:], in1=fb[:],
                                            op=Alu.max)
                for t in range(4):
                    n = g * 4 + t
                    if (m, n) in K_IDX:
                        k = K_IDX[(m, n)]
                        nc.vector.tensor_tensor(
                            out=vs_all[:, k * NT:(k + 1) * NT],
                            in0=pt[:, t * NT:(t + 1) * NT],
                            in1=maskD[:, k * NT:(k + 1) * NT],
                            op=Alu.add)
                        nc.vector.tensor_reduce(
                            mnp[:, k:k + 1],
                            vs_all[:, k * NT:(k + 1) * NT],
                            axis=X, op=Alu.min)
                if g == 0:
                    # row m is complete: finish its max and pos threshold
                    # tp = max_neg + MARGIN = (ln(max_e) + 25)/50 + 0.1; the
                    # pos phase itself is emitted one m-iteration later so the
                    # in-order ACT queue never stalls on this m's DVE chain.
                    nc.vector.reduce_max(mxE[:, m:m + 1], maxacc[m][:], axis=X)
                    nc.scalar.activation(out=lg8[:, m:m + 1],
                                         in_=mxE[:, m:m + 1], func=Act.Ln,
                                         bias=bias_z[:], scale=1.0)
                    nc.vector.tensor_scalar(out=tp8[:, m:m + 1],
                                            in0=lg8[:, m:m + 1],
                                            scalar1=1.0 / SN,
                                            scalar2=THRESH + MARGIN,
                                            op0=Alu.mult, op1=Alu.add)
                    if m > 0:
                        _pos_phase(nc, m - 1, K_IDX, vs_all, tp8, sp,
                                   bias_n, bias_p, posp, Act, Alu, BF16)
        _pos_phase(nc, M_TILES - 1, K_IDX, vs_all, tp8, sp,
                   bias_n, bias_p, posp, Act, Alu, BF16)

        nc.sync.dma_start(outs["mxE"][:], mxE[:])
        nc.sync.dma_start(outs["mnp"][:], mnp[:])
        nc.sync.dma_start(outs["negp"][:], negp[:])
        nc.sync.dma_start(outs["posp"][:], posp[:])


def _numpy_fallback(feats, labels):
    f = np.float32
    sim = feats @ feats.T
    same = labels[:, None] == labels[None, :]
    pos_mask = same & (sim < f(1.0 - EPS))
    neg_mask = ~same
    min_pos = np.where(pos_mask, sim, np.inf).min(axis=1).astype(np.float32)
    max_neg = np.where(neg_mask, sim, -np.inf).max(axis=1).astype(np.float32)
    neg_sel = neg_mask & (sim > (min_pos - f(MARGIN))[:, None])
    pos_sel = pos_mask & (sim < (max_neg + f(MARGIN))[:, None])
    valid = neg_sel.any(axis=1) & pos_sel.any(axis=1)
    ps = np.exp(np.where(pos_sel, -f(SP) * (sim - f(THRESH)), -np.inf),
                dtype=np.float32).sum(axis=1, dtype=np.float32)
    ns = np.exp(np.where(neg_sel, f(SN) * (sim - f(THRESH)), -np.inf),
                dtype=np.float32).sum(axis=1, dtype=np.float32)
    rl = (f(1.0 / SP) * np.log1p(ps) + f(1.0 / SN) * np.log1p(ns)).astype(np.float32)
    loss = np.float32(np.where(valid, rl, f(0)).sum(dtype=np.float32) / f(B))
    prec1 = np.float32(np.mean((1.0 - valid.astype(np.float32)), dtype=np.float32))
    return loss, prec1


def _prep_core_inputs(labs_rot, featsT_rot, bf16):
    fk0 = np.ascontiguousarray(featsT_rot[:P].astype(bf16))
    fk1 = np.ascontiguousarray(featsT_rot[P:].astype(bf16))
    maskD = np.full((P, NK * NT), 30.0, np.float32)
    rowhot = np.zeros((KHOT, NK * P), np.float32)
    colhot = np.zeros((KHOT, NK * NT), np.float32)
    ok = True
    for k, (m, n) in enumerate(COMBOS):
        rows = labs_rot[m * P:(m + 1) * P]
        cols = labs_rot[n * NT:(n + 1) * NT]
        uniq = np.unique(rows)
        if len(uniq) > KHOT:
            ok = False
            break
        rowhot[:len(uniq), k * P:(k + 1) * P] = rows[None, :] == uniq[:, None]
        colhot[:len(uniq), k * NT:(k + 1) * NT] = (
            MASK_NEG * (cols[None, :] == uniq[:, None]))
        r = np.arange(P)
        j = m * P + r - n * NT
        inb = (j >= 0) & (j < NT)
        maskD[r[inb], k * NT + j[inb]] = 60.0
    return ok, {
        "fk0": fk0, "fk1": fk1,
        "maskD": np.ascontiguousarray(maskD.astype(bf16)),
        "rowhot": np.ascontiguousarray(rowhot.astype(bf16)),
        "colhot": np.ascontiguousarray(colhot.astype(bf16)),
    }


def kernel(feats, labels):
    import ml_dtypes
    bf16 = ml_dtypes.bfloat16

    feats = np.ascontiguousarray(np.asarray(feats), dtype=np.float32)
    labels = np.asarray(labels).astype(np.int64).ravel()
    perm = np.argsort(labels, kind="stable")
    labs = labels[perm]
    fs = feats[perm]

    nlab = int(labs.max()) + 1 if labs.size else 1
    counts = np.bincount(labs, minlength=nlab)
    starts = np.cumsum(counts) - counts
    gs_row = starts[labs]
    ge_row = (starts + counts)[labs]
    ok = True
    for c in range(NCORES):
        base = c * SLAB
        for m in range(M_TILES):
            r = slice(base + m * P, base + (m + 1) * P)
            lo, hi = COV[m]
            if (gs_row[r] - base < lo).any() or (ge_row[r] - base > hi).any():
                ok = False
    if not ok:
        return _numpy_fallback(feats, labels)

    from concourse.bass_test_utils import run_kernel
    import concourse.tile as tile

    featsT = np.ascontiguousarray(fs.T)  # [256, 8192]
    ins_list = []
    for c in range(NCORES):
        rot = np.roll(featsT, -c * SLAB, axis=1)
        labr = np.roll(labs, -c * SLAB)
        okc, ins = _prep_core_inputs(labr, rot, bf16)
        if not okc:
            return _numpy_fallback(feats, labels)
        ins_list.append(ins)
    out_like = {
        "mxE": np.zeros((P, M_TILES), np.float32),
        "mnp": np.zeros((P, NK), np.float32),
        "negp": np.zeros((P, 16 * M_TILES), np.float32),
        "posp": np.zeros((P, NK), np.float32),
    }

    res = run_kernel(
        _loss_kernel, None, ins_list, output_like=[out_like] * NCORES,
        bass_type=tile.TileContext, num_cores=NCORES,
        check_with_sim=False, check_with_hw=True, trace_sim=False,
        trace_hw=False,
    )
    global LAST_EXEC_NS, LAST_TRACE
    LAST_EXEC_NS = getattr(res, "exec_time_ns", None)
    LAST_TRACE = getattr(res, "instructions_and_trace", None)

    def grab(cr, key):
        for k, v in cr.items():
            if key in k:
                return np.asarray(v)
        raise KeyError(key)

    f = np.float32
    mxl, mpl, psl, nsl = [], [], [], []
    m_of_k = np.array([m for (m, _) in COMBOS])
    for c in range(NCORES):
        cr = res.results[c]
        mxE = grab(cr, "mxE").astype(np.float32)          # [P, 8]
        mnp = grab(cr, "mnp").astype(np.float32)          # [P, 12]
        negp = grab(cr, "negp").astype(np.float32)        # [P, 128]
        posp = grab(cr, "posp").astype(np.float32)        # [P, 12]
        maxneg = (np.log(mxE) * f(1.0 / SN) + f(THRESH)).T.reshape(-1)
        mnp_m = np.full((P, M_TILES), np.inf, np.float32)
        psp_m = np.zeros((P, M_TILES), np.float32)
        for k in range(NK):
            m = m_of_k[k]
            mnp_m[:, m] = np.minimum(mnp_m[:, m], mnp[:, k])
            psp_m[:, m] += posp[:, k]
        minraw = mnp_m.T.reshape(-1)
        negsum = negp.reshape(P, M_TILES, 16).sum(axis=2,
                                                  dtype=np.float32).T.reshape(-1)
        possum = psp_m.T.reshape(-1)
        mxl.append(maxneg)
        mpl.append(minraw)
        nsl.append(negsum)
        psl.append(possum)

    maxneg = np.concatenate(mxl).astype(np.float32)
    minraw = np.concatenate(mpl).astype(np.float32)
    negsum = np.concatenate(nsl).astype(np.float32)
    possum = np.concatenate(psl).astype(np.float32)
    minpos = np.where(minraw < f(15.0), minraw, np.float32(np.inf)).astype(np.float32)

    tn = minpos - f(MARGIN)
    tp = maxneg + f(MARGIN)
    valid = (maxneg > tn) & (minpos < tp)
    # The kernel's neg LSE is unthresholded: verify the skipped sub-t_n tail
    # is negligible for every valid row, else fall back to exact numpy.
    with np.errstate(over="ignore"):
        leak = f(B) * np.exp(f(SN) * (tn - f(THRESH)), dtype=np.float32)
    bad = valid & ~(leak <= f(1e-6) * negsum)
    if bad.any():
        return _numpy_fallback(feats, labels)

    row_loss = (f(1.0 / SP) * np.log1p(possum)
                + f(1.0 / SN) * np.log1p(negsum)).astype(np.float32)
    loss = np.float32(np.where(valid, row_loss, f(0)).sum(dtype=np.float32) / f(B))
    prec1 = np.float32(np.mean(1.0 - valid.astype(np.float32), dtype=np.float32))
    return loss, prec1


# revision 56
# speedup vs baseline: 1.1415x; 1.1415x over previous
"""Angular-prototypical hard-mining loss on 8 Trainium2 cores (v2).

Host sorts rows by label -> same-label pairs cluster near the diagonal.
Each core gets a 1024-row slab and a column-ROTATED feats^T so its slab is
local columns [0,1024) -> one uniform SPMD program. Per 128-row m-tile,
1-2 aligned 512-col "strip" tiles hold all same-label columns
(host-verified); the rest are pure cross-label.

v2 engine plan (per [128,2048] column group, fp32r matmuls):
  PE:   2 fp32r matmuls per 512-tile into a 4-bank PSUM group; strip tiles
        get a third small bf16 one-hot-label matmul accumulating -30*same
        (includes the diagonal) straight into PSUM -> masked sim with no
        DVE mask work and no value-based diagonal detection.
  ACT:  one Exp over the whole group (scale=50, bias=-25), bf16 out to
        SBUF, fused accum_out = unthresholded neg-LSE partial.
  DVE:  row max via bf16 tensor_tensor(max) chains over the exp outputs
        (2x DVE mode); max_neg recovered on host as (ln(max_e)+25)/50.
        Strips: one tensor_tensor_reduce makes vs = sim + maskD (+30 off
        targets, +60 diag) AND its row min (-> min_pos).
  Pos phase (needs tp = max_neg + margin): one on-device Ln of the 8
        row-max columns, tp8 = lgE/50 + 0.6, then per strip one ACT Exp
        (scale=-2, bias=+1) and one scalar_tensor_tensor
        (vs < tp) * e_v with fused row-sum -> pos partial.

Per-row lgE/min/neg/pos partials go back to the host, which does
valid/log1p/loss/prec1 in f32 (order-invariant -> no un-sort needed).
"""
import os
import sys
import numpy as np

sys.path.insert(0, "/opt/trn_rl_repo")

USE_TTR = os.environ.get("K_TTR", "1") == "1"
USE_STT = os.environ.get("K_STT", "1") == "1"
USE_LN = os.environ.get("K_LN", "1") == "1"
USE_GRPACT = os.environ.get("K_GRPACT", "1") == "1"
USE_HOTMM = os.environ.get("K_HOTMM", "1") == "1"

B, D, NCORES, SLAB = 8192, 256, 8, 1024
P, NT, M_TILES, N_TILES = 128, 512, 8, 16
KHOT = 64
THRESH, MARGIN, SP, SN, EPS = 0.5, 0.1, 2.0, 50.0, 1e-5
MASK_NEG = -30.0   # added to same-label entries (incl diag) for the neg side
BIGV = 1e9
E_MARGIN = 148.4131591025766   # e^(SN*MARGIN): pos threshold in exp domain

LAST_EXEC_NS = None
LAST_TRACE = None

STRIP = {0: (15, 0), 1: (0,), 2: (0,), 3: (0, 1), 4: (0, 1),
         5: (1,), 6: (1,), 7: (1, 2)}
COV = {0: (-512, 512), 1: (0, 512), 2: (0, 512), 3: (0, 1024),
       4: (0, 1024), 5: (512, 1024), 6: (512, 1024), 7: (512, 1536)}
COMBOS = [(m, n) for m in range(M_TILES) for n in STRIP[m]]
NK = len(COMBOS)  # 12
POOL_M = frozenset()  # walrus: Pool supports only memset/DMA, no tensor ops


def _pos_phase(nc, m, K_IDX, vs_all, tp8, sp, bias_n, bias_p, posp, Act, Alu,
               BF16):
    from concourse import mybir
    X = mybir.AxisListType.X
    for n2 in STRIP[m]:
        k = K_IDX[(m, n2)]
        vs = vs_all[:, k * NT:(k + 1) * NT]
        Ev = sp.tile([P, NT], BF16, name=f"Ev{k}", tag="Ev")
        nc.scalar.activation(out=Ev[:], in_=vs, func=Act.Exp,
                             bias=bias_n[:], scale=SN)
        ev = sp.tile([P, NT], BF16, name=f"ev{k}", tag="ev")
        nc.scalar.activation(out=ev[:], in_=vs, func=Act.Exp,
                             bias=bias_p[:], scale=-SP)
        junk = sp.tile([P, NT], BF16, name=f"junk{k}", tag="junk")
        if USE_STT:
            nc.vector.scalar_tensor_tensor(
                out=junk[:], in0=Ev[:], scalar=tp8[:, m:m + 1], in1=ev[:],
                op0=Alu.is_lt, op1=Alu.mult, accum_out=posp[:, k:k + 1])
        else:
            msk = sp.tile([P, NT], BF16, name=f"msk{k}", tag="msk")
            nc.vector.tensor_scalar(out=msk[:], in0=Ev[:],
                                    scalar1=tp8[:, m:m + 1], scalar2=None,
                                    op0=Alu.is_lt)
            nc.vector.tensor_tensor(out=junk[:], in0=msk[:], in1=ev[:],
                                    op=Alu.mult)
            nc.vector.reduce_sum(posp[:, k:k + 1], junk[:], axis=X)


def _loss_kernel(tc, outs, ins):
    from concourse import mybir
    from contextlib import ExitStack

    F32, BF16 = mybir.dt.float32, mybir.dt.bfloat16
    F32R = mybir.dt.float32r
    Alu, Act = mybir.AluOpType, mybir.ActivationFunctionType
    X = mybir.AxisListType.X
    nc = tc.nc
    fk0_d, fk1_d = ins["fk0"], ins["fk1"]
    maskD_d, rowhot_d, colhot_d = ins["maskD"], ins["rowhot"], ins["colhot"]
    K_IDX = {c: i for i, c in enumerate(COMBOS)}

    with ExitStack() as ctx:
        big = ctx.enter_context(tc.tile_pool(name="big", bufs=1))
        ep = ctx.enter_context(tc.tile_pool(name="ep", bufs=3))
        sp = ctx.enter_context(tc.tile_pool(name="sp", bufs=3))
        psp = ctx.enter_context(tc.tile_pool(name="psum", bufs=2, space="PSUM"))

        fk0 = big.tile([P, B], BF16)
        fk1 = big.tile([P, B], BF16)
        maskD = big.tile([P, NK * NT], BF16)
        rowhot = big.tile([KHOT, NK * P], BF16)
        colhot = big.tile([KHOT, NK * NT], BF16)
        vs_all = big.tile([P, NK * NT], F32)
        maxacc = [big.tile([P, NT], BF16, name=f"maxacc{m}", tag=f"mx{m}")
                  for m in range(M_TILES)]
        mxE = big.tile([P, M_TILES], F32)
        tp8 = big.tile([P, M_TILES], F32)
        mnp = big.tile([P, NK], F32)
        negp = big.tile([P, 16 * M_TILES], F32)
        posp = big.tile([P, NK], F32)
        nc.vector.memset(negp[:], 0.0)
        bias_n = big.tile([P, 1], F32)
        bias_p = big.tile([P, 1], F32)
        nc.vector.memset(bias_n[:], -SN * THRESH)
        nc.vector.memset(bias_p[:], SP * THRESH)

        # group 0 (column tiles 0-3) holds 11 of the 12 strips -> process it
        # LAST so the mask DMAs never gate the pipeline start. But cols
        # [0:1024) are every m-tile's stationary operand -> DMA them FIRST.
        G_ORDER = (1, 2, 3, 0)
        CH = 2048
        lhs = slice(0, SLAB)
        nc.sync.dma_start(fk0[:, lhs], fk0_d[:, lhs])
        nc.sync.dma_start(fk1[:, lhs], fk1_d[:, lhs])
        for i in (1, 2, 3):
            cs = slice(i * CH, (i + 1) * CH)
            nc.sync.dma_start(fk0[:, cs], fk0_d[:, cs])
            nc.sync.dma_start(fk1[:, cs], fk1_d[:, cs])
        # masks + trailing g0 columns go down the (idle) gpsimd SWDGE queue,
        # in parallel with the fk stream on the sync queue.
        nc.gpsimd.dma_start(maskD[:], maskD_d[:])
        nc.gpsimd.dma_start(rowhot[:], rowhot_d[:])
        nc.gpsimd.dma_start(colhot[:], colhot_d[:])
        g0r = slice(SLAB, CH)
        nc.gpsimd.dma_start(fk0[:, g0r], fk0_d[:, g0r])
        nc.gpsimd.dma_start(fk1[:, g0r], fk1_d[:, g0r])

        GW = 2048
        for gi, g in enumerate(G_ORDER):
            for m in range(M_TILES):
                mc = slice(m * P, (m + 1) * P)
                pt = psp.tile([P, GW], F32, tag="ps")
                for t in range(4):
                    n = g * 4 + t
                    ncs = slice(n * NT, (n + 1) * NT)
                    sl = pt[:, t * NT:(t + 1) * NT]
                    is_strip = (m, n) in K_IDX
                    nc.tensor.matmul(sl, fk0[:, mc], fk0[:, ncs],
                                     start=True, stop=False)
                    nc.tensor.matmul(sl, fk1[:, mc], fk1[:, ncs],
                                     start=False, stop=not is_strip)
                    if is_strip and USE_HOTMM:
                        k = K_IDX[(m, n)]
                        nc.tensor.matmul(sl, rowhot[:, k * P:(k + 1) * P],
                                         colhot[:, k * NT:(k + 1) * NT],
                                         start=False, stop=True)
                e = ep.tile([P, GW], BF16, tag="e")
                if USE_GRPACT:
                    nc.scalar.activation(
                        out=e[:], in_=pt[:], func=Act.Exp,
                        bias=bias_n[:], scale=SN,
                        accum_out=negp[:, m * 16 + g * 4:m * 16 + g * 4 + 1])
                else:
                    for t in range(4):
                        nc.scalar.activation(
                            out=e[:, t * NT:(t + 1) * NT],
                            in_=pt[:, t * NT:(t + 1) * NT], func=Act.Exp,
                            bias=bias_n[:], scale=SN,
                            accum_out=negp[:, m * 16 + g * 4 + t:
                                           m * 16 + g * 4 + t + 1])
                # row-max chain over the bf16 exps: pairwise tree fold of the
                # group (2x DVE mode on bf16), then max-accumulate into maxacc
                fa = ep.tile([P, 1024], BF16, tag="fa")
                nc.vector.tensor_tensor(out=fa[:], in0=e[:, 0:1024],
                                        in1=e[:, 1024:2048], op=Alu.max)
                if gi == 0:
                    nc.vector.tensor_tensor(out=maxacc[m][:], in0=fa[:, 0:NT],
                                            in1=fa[:, NT:1024], op=Alu.max)
                else:
                    fb = ep.tile([P, NT], BF16, tag="fb")
                    nc.vector.tensor_tensor(out=fb[:], in0=fa[:, 0:NT],
                                            in1=fa[:, NT:1024], op=Alu.max)
                    nc.vector.tensor_tensor(out=maxacc[m][:],
                                            in0=maxacc[m][:], in1=fb[:],
                                            op=Alu.max)
                for t in range(4):
                    n = g * 4 + t
                    if (m, n) in K_IDX:
                        k = K_IDX[(m, n)]
                        nc.vector.tensor_tensor(
                            out=vs_all[:, k * NT:(k + 1) * NT],
                            in0=pt[:, t * NT:(t + 1) * NT],
                            in1=maskD[:, k * NT:(k + 1) * NT],
                            op=Alu.add)
                        nc.vector.tensor_reduce(
                            mnp[:, k:k + 1],
                            vs_all[:, k * NT:(k + 1) * NT],
                            axis=X, op=Alu.min)
                if g == 0:
                    # row m is complete: finish its max and pos threshold
                    # tp = max_neg + MARGIN = (ln(max_e) + 25)/50 + 0.1; the
                    # pos phase itself is emitted one m-iteration later so the
                    # in-order ACT queue never stalls on this m's DVE chain.
                    nc.vector.reduce_max(mxE[:, m:m + 1], maxacc[m][:], axis=X)
                    nc.vector.tensor_scalar(out=tp8[:, m:m + 1],
                                            in0=mxE[:, m:m + 1],
                                            scalar1=E_MARGIN, scalar2=None,
                                            op0=Alu.mult)
                    if m > 0:
                        _pos_phase(nc, m - 1, K_IDX, vs_all, tp8, sp,
                                   bias_n, bias_p, posp, Act, Alu, BF16)
        _pos_phase(nc, M_TILES - 1, K_IDX, vs_all, tp8, sp,
                   bias_n, bias_p, posp, Act, Alu, BF16)

        nc.sync.dma_start(outs["mxE"][:], mxE[:])
        nc.sync.dma_start(outs["mnp"][:], mnp[:])
        nc.sync.dma_start(outs["negp"][:], negp[:])
        nc.sync.dma_start(outs["posp"][:], posp[:])


def _numpy_fallback(feats, labels):
    f = np.float32
    sim = feats @ feats.T
    same = labels[:, None] == labels[None, :]
    pos_mask = same & (sim < f(1.0 - EPS))
    neg_mask = ~same
    min_pos = np.where(pos_mask, sim, np.inf).min(axis=1).astype(np.float32)
    max_neg = np.where(neg_mask, sim, -np.inf).max(axis=1).astype(np.float32)
    neg_sel = neg_mask & (sim > (min_pos - f(MARGIN))[:, None])
    pos_sel = pos_mask & (sim < (max_neg + f(MARGIN))[:, None])
    valid = neg_sel.any(axis=1) & pos_sel.any(axis=1)
    ps = np.exp(np.where(pos_sel, -f(SP) * (sim - f(THRESH)), -np.inf),
                dtype=np.float32).sum(axis=1, dtype=np.float32)
    ns = np.exp(np.where(neg_sel, f(SN) * (sim - f(THRESH)), -np.inf),
                dtype=np.float32).sum(axis=1, dtype=np.float32)
    rl = (f(1.0 / SP) * np.log1p(ps) + f(1.0 / SN) * np.log1p(ns)).astype(np.float32)
    loss = np.float32(np.where(valid, rl, f(0)).sum(dtype=np.float32) / f(B))
    prec1 = np.float32(np.mean((1.0 - valid.astype(np.float32)), dtype=np.float32))
    return loss, prec1


def _prep_core_inputs(labs_rot, featsT_rot, bf16):
    fk0 = np.ascontiguousarray(featsT_rot[:P].astype(bf16))
    fk1 = np.ascontiguousarray(featsT_rot[P:].astype(bf16))
    maskD = np.full((P, NK * NT), 30.0, np.float32)
    rowhot = np.zeros((KHOT, NK * P), np.float32)
    colhot = np.zeros((KHOT, NK * NT), np.float32)
    ok = True
    for k, (m, n) in enumerate(COMBOS):
        rows = labs_rot[m * P:(m + 1) * P]
        cols = labs_rot[n * NT:(n + 1) * NT]
        uniq = np.unique(rows)
        if len(uniq) > KHOT:
            ok = False
            break
        rowhot[:len(uniq), k * P:(k + 1) * P] = rows[None, :] == uniq[:, None]
        colhot[:len(uniq), k * NT:(k + 1) * NT] = (
            MASK_NEG * (cols[None, :] == uniq[:, None]))
        r = np.arange(P)
        j = m * P + r - n * NT
        inb = (j >= 0) & (j < NT)
        maskD[r[inb], k * NT + j[inb]] = 60.0
    return ok, {
        "fk0": fk0, "fk1": fk1,
        "maskD": np.ascontiguousarray(maskD.astype(bf16)),
        "rowhot": np.ascontiguousarray(rowhot.astype(bf16)),
        "colhot": np.ascontiguousarray(colhot.astype(bf16)),
    }


def kernel(feats, labels):
    import ml_dtypes
    bf16 = ml_dtypes.bfloat16

    feats = np.ascontiguousarray(np.asarray(feats), dtype=np.float32)
    labels = np.asarray(labels).astype(np.int64).ravel()
    perm = np.argsort(labels, kind="stable")
    labs = labels[perm]
    fs = feats[perm]

    nlab = int(labs.max()) + 1 if labs.size else 1
    counts = np.bincount(labs, minlength=nlab)
    starts = np.cumsum(counts) - counts
    gs_row = starts[labs]
    ge_row = (starts + counts)[labs]
    ok = True
    for c in range(NCORES):
        base = c * SLAB
        for m in range(M_TILES):
            r = slice(base + m * P, base + (m + 1) * P)
            lo, hi = COV[m]
            if (gs_row[r] - base < lo).any() or (ge_row[r] - base > hi).any():
                ok = False
    if not ok:
        return _numpy_fallback(feats, labels)

    from concourse.bass_test_utils import run_kernel
    import concourse.tile as tile

    featsT = np.ascontiguousarray(fs.T)  # [256, 8192]
    ins_list = []
    for c in range(NCORES):
        rot = np.roll(featsT, -c * SLAB, axis=1)
        labr = np.roll(labs, -c * SLAB)
        okc, ins = _prep_core_inputs(labr, rot, bf16)
        if not okc:
            return _numpy_fallback(feats, labels)
        ins_list.append(ins)
    out_like = {
        "mxE": np.zeros((P, M_TILES), np.float32),
        "mnp": np.zeros((P, NK), np.float32),
        "negp": np.zeros((P, 16 * M_TILES), np.float32),
        "posp": np.zeros((P, NK), np.float32),
    }

    res = run_kernel(
        _loss_kernel, None, ins_list, output_like=[out_like] * NCORES,
        bass_type=tile.TileContext, num_cores=NCORES,
        check_with_sim=False, check_with_hw=True, trace_sim=False,
        trace_hw=False,
    )
    global LAST_EXEC_NS, LAST_TRACE
    LAST_EXEC_NS = getattr(res, "exec_time_ns", None)
    LAST_TRACE = getattr(res, "instructions_and_trace", None)

    def grab(cr, key):
        for k, v in cr.items():
            if key in k:
                return np.asarray(v)
        raise KeyError(key)

    f = np.float32
    mxl, mpl, psl, nsl = [], [], [], []
    m_of_k = np.array([m for (m, _) in COMBOS])
    for c in range(NCORES):
        cr = res.results[c]
        mxE = grab(cr, "mxE").astype(np.float32)          # [P, 8]
        mnp = grab(cr, "mnp").astype(np.float32)          # [P, 12]
        negp = grab(cr, "negp").astype(np.float32)        # [P, 128]
        posp = grab(cr, "posp").astype(np.float32)        # [P, 12]
        maxneg = (np.log(mxE) * f(1.0 / SN) + f(THRESH)).T.reshape(-1)
        mnp_m = np.full((P, M_TILES), np.inf, np.float32)
        psp_m = np.zeros((P, M_TILES), np.float32)
        for k in range(NK):
            m = m_of_k[k]
            mnp_m[:, m] = np.minimum(mnp_m[:, m], mnp[:, k])
            psp_m[:, m] += posp[:, k]
        minraw = mnp_m.T.reshape(-1)
        negsum = negp.reshape(P, M_TILES, 16).sum(axis=2,
                                                  dtype=np.float32).T.reshape(-1)
        possum = psp_m.T.reshape(-1)
        mxl.append(maxneg)
        mpl.append(minraw)
        nsl.append(negsum)
        psl.append(possum)

    maxneg = np.concatenate(mxl).astype(np.float32)
    minraw = np.concatenate(mpl).astype(np.float32)
    negsum = np.concatenate(nsl).astype(np.float32)
    possum = np.concatenate(psl).astype(np.float32)
    minpos = np.where(minraw < f(15.0), minraw, np.float32(np.inf)).astype(np.float32)

    tn = minpos - f(MARGIN)
    tp = maxneg + f(MARGIN)
    valid = (maxneg > tn) & (minpos < tp)
    # The kernel's neg LSE is unthresholded: verify the skipped sub-t_n tail
    # is negligible for every valid row, else fall back to exact numpy.
    with np.errstate(over="ignore"):
        leak = f(B) * np.exp(f(SN) * (tn - f(THRESH)), dtype=np.float32)
    bad = valid & ~(leak <= f(1e-6) * negsum)
    if bad.any():
        return _numpy_fallback(feats, labels)

    row_loss = (f(1.0 / SP) * np.log1p(possum)
                + f(1.0 / SN) * np.log1p(negsum)).astype(np.float32)
    loss = np.float32(np.where(valid, row_loss, f(0)).sum(dtype=np.float32) / f(B))
    prec1 = np.float32(np.mean(1.0 - valid.astype(np.float32), dtype=np.float32))
    return loss, prec1
